# revision 14
# baseline (speedup 1.0000x reference)
"""BinaryTreeRNN Trainium2 kernel — 8-core data-parallel, v5.

Contract: kernel(**inputs) takes FULL unsharded inputs (x [4M,16] f32 plus tiny
tree params) and returns the FULL [4M] f32 output.

Design (per core, N_core = 500k samples, padded to 501760 = 560 blocks x 896):
  * Host folds tree params in float64:  softmax(om) -> per-node (A, B, P, R,
    phi);  S*sin(s)+C*cos(s) == R*sin(s+phi).  gamma = 1/2pi is folded into
    every matmul column and inter-level scale so range reduction needs no
    multiplies:  each level's hidden outputs are emitted as o_hat = gamma*o.
  * Host packs x to fp16 [n_st, 113, B*128] stationary tiles: rows 16a+v
    (a<7) hold x[blk*896 + 7p + a, v] in column p; row 112 holds 1.0 (bias).
    fp16 x + fp16 W give ~1.5e-3 L2 error (gate 2e-2) and a SINGLE fp16
    matmul per block: out cols (28 each) = l''(= gamma*P3*leaf_l), r(leaf_r),
    sc3 (= gamma*(l + r + phi3), sin-ready).
  * Per seg (supertile of 16 blocks), from PSUM: p3'' = l''*r (DVE TT),
    sc3 copy to SBUF (ACT).  Group-batched (GROUP=5 supertiles):
      k = Ident(sc + MAGIC) [ACT]          -> round-to-nearest lattice
      f = (k - MAGIC) - sc  [DVE STT]      -> -frac residual
      t = Sin(f * -2pi)     [ACT]          == sin(2pi*sc)
      w = (sc*A + beta) + p''  [DVE affine_then_add]
      o_hat = (t * gamma*R) + w  [Pool STT]
    Levels 2/1: sc/p'' via Pool STTs on children (phase + P/gamma^k folded
    into the scalar slots), same k/f/t/w/o chain.  y stored from level 1.
"""

import os
import sys

for _p in ("/opt/trn_rl_repo", "/root/.axon_site/_ro/trn_rl_repo"):
    if os.path.isdir(_p) and _p not in sys.path:
        sys.path.append(_p)

import numpy as np

N_FULL = 4_000_000
V = 16
N_CORES = 8
N_CORE = N_FULL // N_CORES          # 500_000
SLOTS = 7                            # samples per stationary column
BLK = 128 * SLOTS                    # 896 samples per matmul block
N_BLOCKS = 560                       # ceil(500000/896) -> padded
N_PAD = N_BLOCKS * BLK               # 501_760
B = 16                               # blocks per super-tile
N_ST = N_BLOCKS // B                 # 35
KROWS = 128                          # 112 data rows + 16 ones rows (DMA pads
                                     # to full 128 partitions: non-128 partition
                                     # dims serialize onto one DMA engine)

MAGIC = float(np.float32(1.5 * 2**23))
GAMMA = 1.0 / (2.0 * np.pi)
NEG2PI = float(np.float32(-2.0 * np.pi))

F32 = np.float32


def _softmax64(om):
    e = np.exp(om.astype(np.float64) - om.astype(np.float64).max(-1, keepdims=True))
    return e / e.sum(-1, keepdims=True)


def _fold(leaf_w, leaf_b, w1, b1, om1, w2, b2, om2, w3, b3, om3):
    """float64 constant folding. Returns per-level scalar dicts + wp [113,84]."""
    lv = {}
    for lvl, (w, b, om) in {3: (w3, b3, om3), 2: (w2, b2, om2), 1: (w1, b1, om1)}.items():
        sm = _softmax64(om)
        w64 = w.astype(np.float64)
        lv[lvl] = dict(
            A=w64 * sm[:, 0], S=w64 * sm[:, 1], C=w64 * sm[:, 2], P=w64 * sm[:, 3],
            B=b.astype(np.float64),
        )
        lv[lvl]["R"] = np.hypot(lv[lvl]["S"], lv[lvl]["C"])
        lv[lvl]["phi"] = np.arctan2(lv[lvl]["C"], lv[lvl]["S"])

    g = GAMMA
    c3, c2, c1 = lv[3], lv[2], lv[1]

    def f(v):
        return float(F32(v))

    # Per-node op scalars.  Level-3/2 outputs are emitted scaled by gamma.
    # L3: phi3 lives inside the sc column; beta compensates the A*phi shift.
    # Parent phases ride as equal offsets delta = gamma*phi/2 folded into both
    # children's betas; the resulting product pollution delta*gamma*s is
    # compensated in the parent's linear coefficient (At = A - P*delta/gamma).
    d2 = [g * c2["phi"][m] / 2 for m in range(2)]
    d1 = g * c1["phi"][0] / 2
    At2 = [c2["A"][m] - c2["P"][m] * d2[m] / g for m in range(2)]
    At1 = c1["A"][0] / g - c1["P"][0] * d1 / g ** 2

    L3 = [dict(A=f(c3["A"][n]),
               beta=f(g * (c3["B"][n] - c3["A"][n] * c3["phi"][n]) + d2[n // 2]),
               gR=f(g * c3["R"][n])) for n in range(4)]
    L2 = [dict(A=f(At2[m]),
               beta=f(g * c2["B"][m] - At2[m] * g * c2["phi"][m]
                      - c2["P"][m] * d2[m] ** 2 / g + d1),
               gR=f(g * c2["R"][m]), Pg=f(c2["P"][m] / g)) for m in range(2)]
    L1 = [dict(A=f(At1),
               beta=f(c1["B"][0] - At1 * g * c1["phi"][0]
                      - c1["P"][0] * d1 ** 2 / g ** 2),
               gR=f(c1["R"][0]), Pg=f(c1["P"][0] / g ** 2))]

    # wp [113, 84]: col 28j + 7n + a; j: 0=l''(gamma*P3-premult), 1=r, 2=sc3.
    # rows 16a+v = weight of x[., v] for slot a; row 112 = bias (ones row).
    wp = np.zeros((KROWS, 84), np.float64)
    lw = leaf_w.astype(np.float64)
    lb = leaf_b.astype(np.float64)
    for n in range(4):
        wl, bl = lw[2 * n], lb[2 * n]
        wr, br = lw[2 * n + 1], lb[2 * n + 1]
        cl = g * c3["P"][n]
        funcs = [(0, cl * wl, cl * bl), (1, wr, br),
                 (2, g * (wl + wr), g * (bl + br + c3["phi"][n]))]
        for j, wv, bias in funcs:
            for a in range(SLOTS):
                wp[16 * a: 16 * a + 16, 28 * j + 7 * n + a] = wv
                wp[112, 28 * j + 7 * n + a] = bias
    return L3, L2, L1, wp.astype(np.float16)


def _pack_x(x_shard, n_st=N_ST, b_blocks=B):
    """[n, 16] f32 -> fp16 [n_st, 113, b_blocks*128] stationary tiles."""
    npad = n_st * b_blocks * BLK
    xs = np.empty((npad, V), F32)
    xs[:len(x_shard)] = x_shard
    xs[len(x_shard):] = 1.0
    a = xs.reshape(n_st, b_blocks, 128, SLOTS, V)  # [st, b, p, a, v]
    xt = np.empty((n_st, KROWS, b_blocks, 128), F32)
    xt[:, :112] = a.transpose(0, 3, 4, 1, 2).reshape(n_st, 112, b_blocks, 128)
    xt[:, 112:] = 1.0
    return xt.reshape(n_st, KROWS, b_blocks * 128).astype(np.float16)


_PROGRAM_CACHE = {}


def _build_program(n_st=N_ST, b_blocks=B):
    """Build + compile the per-core Bass program (identical on all cores)."""
    import json
    key = (n_st, b_blocks, json.dumps(_build_program.consts, sort_keys=True, default=str))
    if key in _PROGRAM_CACHE:
        return _PROGRAM_CACHE[key]

    import concourse.bass as bass
    import concourse.tile as tile
    from concourse import bacc, mybir
    from contextlib import ExitStack

    f32 = mybir.dt.float32
    f16 = mybir.dt.float16
    Sin = mybir.ActivationFunctionType.Sin
    Ident = mybir.ActivationFunctionType.Identity
    sub = mybir.AluOpType.subtract
    mult = mybir.AluOpType.mult
    addop = mybir.AluOpType.add

    L3, L2, L1 = _build_program.consts
    GROUP = 5

    nc = bacc.Bacc("TRN2", target_bir_lowering=False, debug=False,
                   num_devices=N_CORES)
    xh_d = nc.dram_tensor("xh", [n_st, KROWS, b_blocks * 128], f16,
                          kind="ExternalInput")
    wp_d = nc.dram_tensor("wp", [KROWS, 84], f16, kind="ExternalInput")
    out_d = nc.dram_tensor("out", [n_st, 128, b_blocks, SLOTS], f32,
                           kind="ExternalOutput")

    # Activation float biases require pre-registered const APs.
    def reg_const(v):
        if (f32, v) not in nc.const_aps.aps:
            t = nc.alloc_sbuf_tensor(
                f"constx-{len(nc.const_aps.aps)}", [128, 1], f32)
            nc.gpsimd.memset(t.ap(), v)
            nc.const_aps.aps[(f32, v)] = t.ap()

    reg_const(MAGIC)
    reg_const(0.0)
    nc.all_engine_barrier()

    with tile.TileContext(nc) as tc:
        with ExitStack() as ctx:
            const_pool = ctx.enter_context(tc.tile_pool(name="const", bufs=1))
            xpool = ctx.enter_context(tc.tile_pool(name="x", bufs=2))
            ppool = ctx.enter_context(
                tc.tile_pool(name="ps", bufs=2, space=bass.MemorySpace.PSUM))
            gpool = ctx.enter_context(tc.tile_pool(name="g", bufs=2))

            wp = const_pool.tile([KROWS, 84], f16)
            nc.sync.dma_start(wp[:], wp_d[:])

            st0 = 0
            while st0 < n_st:
                glen = min(GROUP, n_st - st0)
                q = glen * b_blocks
                ccols = {"rsg": 56, "p3g": 28, "k3g": 28, "f3g": 28,
                         "t3g": 28, "w3g": 28, "o3g": 28,
                         "sc2": 14, "p2": 14, "k2": 14, "f2": 14,
                         "t2": 14, "w2": 14, "o2": 14,
                         "sc1": 7, "p1": 7, "k1": 7, "f1": 7,
                         "t1": 7, "w1": 7, "yo": 7}

                def gt(nm, bufs=1):
                    c = ccols[nm]
                    t = gpool.tile([128, glen * b_blocks * c], f32,
                                   name=nm, tag=nm, bufs=bufs)
                    return t, t[:].rearrange("p (q c) -> p q c", c=c)

                rsg, rsgv = gt("rsg", bufs=2)
                p3g, p3gv = gt("p3g", bufs=2)
                sc3v = rsgv[:, :, 28:56]

                for seg in range(glen):
                    st = st0 + seg
                    x2h = xpool.tile([KROWS, b_blocks * 128], f16, name="x2h",
                                     tag="x2h")
                    nc.sync.dma_start(x2h[:], xh_d[st])

                    ps = ppool.tile([128, b_blocks * 128], f32)
                    for b in range(b_blocks):
                        nc.tensor.matmul(ps[:, 128 * b:128 * b + 84],
                                         x2h[:, 128 * b:128 * b + 128],
                                         wp[:], start=True, stop=True)
                    psv = ps[:].rearrange("p (b c) -> p b c", c=128)
                    segsl = slice(seg * b_blocks, (seg + 1) * b_blocks)

                    # stage r + sc3 (one ACT copy); p3'' = l''(PSUM) * r(SBUF)
                    nc.scalar.copy(rsgv[:, segsl, :], psv[:, :, 28:84])
                    nc.vector.tensor_mul(p3gv[:, segsl, :],
                                         psv[:, :, 0:28],
                                         rsgv[:, segsl, 0:28])

                # ---- level 3 (batched over the group) ----
                qf28 = q * 28
                sc3q = sc3v[:, 0:q, :]
                k3g, k3gv = gt("k3g")
                nc.scalar.activation(k3gv[:, 0:q, :], sc3q, Ident,
                                     bias=MAGIC, scale=1.0)
                f3g, f3gv = gt("f3g")
                nc.vector.scalar_tensor_tensor(f3gv[:, 0:q, :],
                                               k3gv[:, 0:q, :],
                                               MAGIC, sc3q, sub, sub)
                t3g, t3gv = gt("t3g", bufs=2)
                nc.scalar.activation(t3g[:, 0:qf28], f3g[:, 0:qf28], Sin,
                                     bias=0.0, scale=NEG2PI)
                w3g, w3gv = gt("w3g", bufs=2)
                o3g, o3gv = gt("o3g", bufs=2)
                for n in range(4):
                    cn = L3[n]
                    sl = (slice(None), slice(0, q), slice(7 * n, 7 * n + 7))
                    scsl = (slice(None), slice(0, q),
                            slice(28 + 7 * n, 28 + 7 * n + 7))
                    nc.vector.affine_then_add(w3gv[sl], rsgv[scsl], p3gv[sl],
                                              scale=cn["A"], bias=cn["beta"])
                    nc.vector.scalar_tensor_tensor(
                        o3gv[sl], t3gv[sl], cn["gR"], w3gv[sl], mult, addop)

                # ---- level 2 ----
                # children of node m are o3 cols (2m, 2m+1).  L2/L1 tensors
                # are m-major ([m, q, 7] flat) so per-m slices are contiguous
                # (strided TENSOR_SCALAR is 4x slower and loses 2x mode).
                qf14 = q * 14
                qf7 = q * 7
                l2in = o3g[:].rearrange("p (q m c) -> p m q c", m=2, c=14)
                asl2 = (slice(None), slice(None), slice(0, q), slice(0, 7))
                bsl2 = (slice(None), slice(None), slice(0, q), slice(7, 14))
                sc2, sc2v = gt("sc2")
                sc2m = sc2[:].rearrange("p (m q c) -> p m q c", m=2, c=7)
                nc.gpsimd.tensor_add(sc2m, l2in[asl2], l2in[bsl2])
                p2, p2v = gt("p2")
                p2m = p2[:].rearrange("p (m q c) -> p m q c", m=2, c=7)
                nc.gpsimd.tensor_mul(p2m, l2in[asl2], l2in[bsl2])
                k2, _ = gt("k2")
                nc.scalar.activation(k2[:, 0:qf14], sc2[:, 0:qf14], Ident,
                                     bias=MAGIC, scale=1.0)
                f2, _ = gt("f2")
                nc.vector.scalar_tensor_tensor(f2[:, 0:qf14], k2[:, 0:qf14],
                                               MAGIC, sc2[:, 0:qf14], sub, sub)
                t2, t2v = gt("t2")
                nc.scalar.activation(t2[:, 0:qf14], f2[:, 0:qf14], Sin,
                                     bias=0.0, scale=NEG2PI)
                w2, w2v = gt("w2")
                o2, o2v = gt("o2")
                for m in range(2):
                    cm = L2[m]
                    sl = slice(m * qf7, (m + 1) * qf7)
                    nc.vector.tensor_scalar(w2[:, sl], sc2[:, sl], cm["A"],
                                            cm["beta"], mult, addop)
                    nc.vector.scalar_tensor_tensor(
                        w2[:, sl], p2[:, sl], cm["Pg"], w2[:, sl], mult, addop)
                    nc.vector.scalar_tensor_tensor(
                        o2[:, sl], t2[:, sl], cm["gR"], w2[:, sl], mult, addop)

                # ---- level 1 ----
                c1 = L1[0]
                sc1, sc1v = gt("sc1")
                nc.gpsimd.tensor_add(sc1[:, 0:qf7], o2[:, 0:qf7],
                                     o2[:, qf7:2 * qf7])
                p1, p1v = gt("p1")
                nc.gpsimd.tensor_mul(p1[:, 0:qf7], o2[:, 0:qf7],
                                     o2[:, qf7:2 * qf7])
                k1, _ = gt("k1")
                nc.scalar.activation(k1[:, 0:qf7], sc1[:, 0:qf7], Ident,
                                     bias=MAGIC, scale=1.0)
                f1, _ = gt("f1")
                nc.vector.scalar_tensor_tensor(f1[:, 0:qf7], k1[:, 0:qf7],
                                               MAGIC, sc1[:, 0:qf7], sub, sub)
                t1, t1v = gt("t1")
                nc.scalar.activation(t1[:, 0:qf7], f1[:, 0:qf7], Sin,
                                     bias=0.0, scale=NEG2PI)
                w1, w1v = gt("w1")
                nc.vector.tensor_scalar(w1[:, 0:qf7], sc1[:, 0:qf7], c1["A"],
                                        c1["beta"], mult, addop)
                nc.vector.scalar_tensor_tensor(
                    w1[:, 0:qf7], p1[:, 0:qf7], c1["Pg"], w1[:, 0:qf7],
                    mult, addop)
                yo, yov = gt("yo")
                nc.vector.scalar_tensor_tensor(
                    yo[:, 0:qf7], t1[:, 0:qf7], c1["gR"], w1[:, 0:qf7],
                    mult, addop)

                dst = out_d[st0:st0 + glen].transpose([1, 0, 2, 3])
                yo4 = yo[:, 0:qf7].rearrange("p (g b a) -> p g b a",
                                             g=glen, a=SLOTS)
                nc.sync.dma_start(dst, yo4)
                st0 += glen

    nc.compile()
    _PROGRAM_CACHE[key] = nc
    return nc


def kernel(x, leaf_w, leaf_b, w1, b1, om1, w2, b2, om2, w3, b3, om3):
    from concourse.bass_interp import get_hw_module
    from concourse.bass_utils import run_bass_kernel_spmd

    L3, L2, L1, wp = _fold(leaf_w, leaf_b, w1, b1, om1, w2, b2, om2, w3, b3, om3)
    _build_program.consts = (L3, L2, L1)
    nc = _build_program()

    in_maps = []
    x = np.ascontiguousarray(x, dtype=F32)
    for c in range(N_CORES):
        xh = _pack_x(x[c * N_CORE:(c + 1) * N_CORE])
        in_maps.append({"xh": xh, "wp": wp})

    kw = {}
    if os.environ.get("KERNEL_TRACE_DIR"):
        kw["tmpdir"] = os.environ["KERNEL_TRACE_DIR"]
    old = nc.m
    nc.m = get_hw_module(nc.m)
    try:
        res = run_bass_kernel_spmd(nc, in_maps, core_ids=list(range(N_CORES)), **kw)
    finally:
        nc.m = old
    kernel._last = res

    out = np.empty(N_FULL, F32)
    for c in range(N_CORES):
        oc = res.results[c]["out"]          # [N_ST, 128, B, 7]
        oc = oc.transpose(0, 2, 1, 3).reshape(-1)[:N_CORE]
        out[c * N_CORE:(c + 1) * N_CORE] = oc
    return out


# revision 15
# speedup vs baseline: 1.0170x; 1.0170x over previous
"""BinaryTreeRNN Trainium2 kernel — 8-core data-parallel, v5.

Contract: kernel(**inputs) takes FULL unsharded inputs (x [4M,16] f32 plus tiny
tree params) and returns the FULL [4M] f32 output.

Design (per core, N_core = 500k samples, padded to 501760 = 560 blocks x 896):
  * Host folds tree params in float64:  softmax(om) -> per-node (A, B, P, R,
    phi);  S*sin(s)+C*cos(s) == R*sin(s+phi).  gamma = 1/2pi is folded into
    every matmul column and inter-level scale so range reduction needs no
    multiplies:  each level's hidden outputs are emitted as o_hat = gamma*o.
  * Host packs x to fp16 [n_st, 113, B*128] stationary tiles: rows 16a+v
    (a<7) hold x[blk*896 + 7p + a, v] in column p; row 112 holds 1.0 (bias).
    fp16 x + fp16 W give ~1.5e-3 L2 error (gate 2e-2) and a SINGLE fp16
    matmul per block: out cols (28 each) = l''(= gamma*P3*leaf_l), r(leaf_r),
    sc3 (= gamma*(l + r + phi3), sin-ready).
  * Per seg (supertile of 16 blocks), from PSUM: p3'' = l''*r (DVE TT),
    sc3 copy to SBUF (ACT).  Group-batched (GROUP=5 supertiles):
      k = Ident(sc + MAGIC) [ACT]          -> round-to-nearest lattice
      f = (k - MAGIC) - sc  [DVE STT]      -> -frac residual
      t = Sin(f * -2pi)     [ACT]          == sin(2pi*sc)
      w = (sc*A + beta) + p''  [DVE affine_then_add]
      o_hat = (t * gamma*R) + w  [Pool STT]
    Levels 2/1: sc/p'' via Pool STTs on children (phase + P/gamma^k folded
    into the scalar slots), same k/f/t/w/o chain.  y stored from level 1.
"""

import os
import sys

for _p in ("/opt/trn_rl_repo", "/root/.axon_site/_ro/trn_rl_repo"):
    if os.path.isdir(_p) and _p not in sys.path:
        sys.path.append(_p)

import numpy as np

N_FULL = 4_000_000
V = 16
N_CORES = 8
N_CORE = N_FULL // N_CORES          # 500_000
SLOTS = 7                            # samples per stationary column
BLK = 128 * SLOTS                    # 896 samples per matmul block
N_BLOCKS = 560                       # ceil(500000/896) -> padded
N_PAD = N_BLOCKS * BLK               # 501_760
B = 16                               # blocks per super-tile
N_ST = N_BLOCKS // B                 # 35
KROWS = 128                          # 112 data rows + 16 ones rows (DMA pads
                                     # to full 128 partitions: non-128 partition
                                     # dims serialize onto one DMA engine)

MAGIC = float(np.float32(1.5 * 2**23))
GAMMA = 1.0 / (2.0 * np.pi)
NEG2PI = float(np.float32(-2.0 * np.pi))

F32 = np.float32


def _softmax64(om):
    e = np.exp(om.astype(np.float64) - om.astype(np.float64).max(-1, keepdims=True))
    return e / e.sum(-1, keepdims=True)


def _fold(leaf_w, leaf_b, w1, b1, om1, w2, b2, om2, w3, b3, om3):
    """float64 constant folding. Returns per-level scalar dicts + wp [113,84]."""
    lv = {}
    for lvl, (w, b, om) in {3: (w3, b3, om3), 2: (w2, b2, om2), 1: (w1, b1, om1)}.items():
        sm = _softmax64(om)
        w64 = w.astype(np.float64)
        lv[lvl] = dict(
            A=w64 * sm[:, 0], S=w64 * sm[:, 1], C=w64 * sm[:, 2], P=w64 * sm[:, 3],
            B=b.astype(np.float64),
        )
        lv[lvl]["R"] = np.hypot(lv[lvl]["S"], lv[lvl]["C"])
        lv[lvl]["phi"] = np.arctan2(lv[lvl]["C"], lv[lvl]["S"])

    g = GAMMA
    c3, c2, c1 = lv[3], lv[2], lv[1]

    def f(v):
        return float(F32(v))

    # Per-node op scalars.  Level-3/2 outputs are emitted scaled by gamma.
    # L3: phi3 lives inside the sc column; beta compensates the A*phi shift.
    # Parent phases ride as equal offsets delta = gamma*phi/2 folded into both
    # children's betas; the resulting product pollution delta*gamma*s is
    # compensated in the parent's linear coefficient (At = A - P*delta/gamma).
    d2 = [g * c2["phi"][m] / 2 for m in range(2)]
    d1 = g * c1["phi"][0] / 2
    At2 = [c2["A"][m] - c2["P"][m] * d2[m] / g for m in range(2)]
    At1 = c1["A"][0] / g - c1["P"][0] * d1 / g ** 2

    L3 = [dict(A=f(c3["A"][n]),
               beta=f(g * (c3["B"][n] - c3["A"][n] * c3["phi"][n]) + d2[n // 2]),
               gR=f(g * c3["R"][n])) for n in range(4)]
    L2 = [dict(A=f(At2[m]),
               beta=f(g * c2["B"][m] - At2[m] * g * c2["phi"][m]
                      - c2["P"][m] * d2[m] ** 2 / g + d1),
               gR=f(g * c2["R"][m]), Pg=f(c2["P"][m] / g)) for m in range(2)]
    L1 = [dict(A=f(At1),
               beta=f(c1["B"][0] - At1 * g * c1["phi"][0]
                      - c1["P"][0] * d1 ** 2 / g ** 2),
               gR=f(c1["R"][0]), Pg=f(c1["P"][0] / g ** 2))]

    # wp [113, 84]: col 28j + 7n + a; j: 0=l''(gamma*P3-premult), 1=r, 2=sc3.
    # rows 16a+v = weight of x[., v] for slot a; row 112 = bias (ones row).
    wp = np.zeros((KROWS, 84), np.float64)
    lw = leaf_w.astype(np.float64)
    lb = leaf_b.astype(np.float64)
    for n in range(4):
        wl, bl = lw[2 * n], lb[2 * n]
        wr, br = lw[2 * n + 1], lb[2 * n + 1]
        cl = g * c3["P"][n]
        funcs = [(0, cl * wl, cl * bl), (1, wr, br),
                 (2, g * (wl + wr), g * (bl + br + c3["phi"][n]))]
        for j, wv, bias in funcs:
            for a in range(SLOTS):
                wp[16 * a: 16 * a + 16, 28 * j + 7 * n + a] = wv
                wp[112, 28 * j + 7 * n + a] = bias
    return L3, L2, L1, wp.astype(np.float16)


def _pack_x(x_shard, n_st=N_ST, b_blocks=B):
    """[n, 16] f32 -> fp16 [n_st, 113, b_blocks*128] stationary tiles."""
    npad = n_st * b_blocks * BLK
    xs = np.empty((npad, V), F32)
    xs[:len(x_shard)] = x_shard
    xs[len(x_shard):] = 1.0
    a = xs.reshape(n_st, b_blocks, 128, SLOTS, V)  # [st, b, p, a, v]
    xt = np.empty((n_st, KROWS, b_blocks, 128), F32)
    xt[:, :112] = a.transpose(0, 3, 4, 1, 2).reshape(n_st, 112, b_blocks, 128)
    xt[:, 112:] = 1.0
    return xt.reshape(n_st, KROWS, b_blocks * 128).astype(np.float16)


_PROGRAM_CACHE = {}


def _build_program(n_st=N_ST, b_blocks=B):
    """Build + compile the per-core Bass program (identical on all cores)."""
    import json
    key = (n_st, b_blocks, json.dumps(_build_program.consts, sort_keys=True, default=str))
    if key in _PROGRAM_CACHE:
        return _PROGRAM_CACHE[key]

    import concourse.bass as bass
    import concourse.tile as tile
    from concourse import bacc, mybir
    from contextlib import ExitStack

    f32 = mybir.dt.float32
    f16 = mybir.dt.float16
    Sin = mybir.ActivationFunctionType.Sin
    Ident = mybir.ActivationFunctionType.Identity
    sub = mybir.AluOpType.subtract
    mult = mybir.AluOpType.mult
    addop = mybir.AluOpType.add

    L3, L2, L1 = _build_program.consts
    GROUP = 5

    nc = bacc.Bacc("TRN2", target_bir_lowering=False, debug=False,
                   num_devices=N_CORES)
    xh_d = nc.dram_tensor("xh", [n_st, KROWS, b_blocks * 128], f16,
                          kind="ExternalInput")
    wp_d = nc.dram_tensor("wp", [KROWS, 84], f16, kind="ExternalInput")
    out_d = nc.dram_tensor("out", [n_st, 128, b_blocks, SLOTS], f32,
                           kind="ExternalOutput")

    # Activation float biases require pre-registered const APs.
    def reg_const(v):
        if (f32, v) not in nc.const_aps.aps:
            t = nc.alloc_sbuf_tensor(
                f"constx-{len(nc.const_aps.aps)}", [128, 1], f32)
            nc.gpsimd.memset(t.ap(), v)
            nc.const_aps.aps[(f32, v)] = t.ap()

    reg_const(MAGIC)
    reg_const(0.0)
    nc.all_engine_barrier()

    with tile.TileContext(nc) as tc:
        with ExitStack() as ctx:
            const_pool = ctx.enter_context(tc.tile_pool(name="const", bufs=1))
            xpool = ctx.enter_context(tc.tile_pool(name="x", bufs=2))
            ppool = ctx.enter_context(
                tc.tile_pool(name="ps", bufs=2, space=bass.MemorySpace.PSUM))
            gpool = ctx.enter_context(tc.tile_pool(name="g", bufs=2))

            wp = const_pool.tile([KROWS, 84], f16)
            nc.sync.dma_start(wp[:], wp_d[:])

            st0 = 0
            while st0 < n_st:
                glen = min(GROUP, n_st - st0)
                q = glen * b_blocks
                ccols = {"rsg": 56, "p3g": 28, "k3g": 28, "f3g": 28,
                         "t3g": 28, "w3g": 28, "o3g": 28,
                         "sc2": 14, "p2": 14, "k2": 14, "f2": 14,
                         "t2": 14, "w2": 14, "o2": 14,
                         "sc1": 7, "p1": 7, "k1": 7, "f1": 7,
                         "t1": 7, "w1": 7, "yo": 7}

                def gt(nm, bufs=1):
                    c = ccols[nm]
                    t = gpool.tile([128, glen * b_blocks * c], f32,
                                   name=nm, tag=nm, bufs=bufs)
                    return t, t[:].rearrange("p (q c) -> p q c", c=c)

                rsg, rsgv = gt("rsg", bufs=2)
                p3g, p3gv = gt("p3g", bufs=2)
                sc3v = rsgv[:, :, 28:56]

                for seg in range(glen):
                    st = st0 + seg
                    x2h = xpool.tile([KROWS, b_blocks * 128], f16, name="x2h",
                                     tag="x2h")
                    nc.sync.dma_start(x2h[:], xh_d[st])

                    ps = ppool.tile([128, b_blocks * 128], f32)
                    for b in range(b_blocks):
                        nc.tensor.matmul(ps[:, 128 * b:128 * b + 84],
                                         x2h[:, 128 * b:128 * b + 128],
                                         wp[:], start=True, stop=True)
                    psv = ps[:].rearrange("p (b c) -> p b c", c=128)
                    segsl = slice(seg * b_blocks, (seg + 1) * b_blocks)

                    # stage r + sc3 (one ACT copy); p3'' = l''(PSUM) * r(SBUF)
                    nc.scalar.copy(rsgv[:, segsl, :], psv[:, :, 28:84])
                    nc.vector.tensor_mul(p3gv[:, segsl, :],
                                         psv[:, :, 0:28],
                                         rsgv[:, segsl, 0:28])

                # ---- level 3 (batched over the group) ----
                qf28 = q * 28
                sc3q = sc3v[:, 0:q, :]
                k3g, k3gv = gt("k3g")
                nc.scalar.activation(k3gv[:, 0:q, :], sc3q, Ident,
                                     bias=MAGIC, scale=1.0)
                f3g, f3gv = gt("f3g")
                nc.vector.scalar_tensor_tensor(f3gv[:, 0:q, :],
                                               k3gv[:, 0:q, :],
                                               MAGIC, sc3q, sub, sub)
                t3g, t3gv = gt("t3g", bufs=2)
                nc.scalar.activation(t3g[:, 0:qf28], f3g[:, 0:qf28], Sin,
                                     bias=0.0, scale=NEG2PI)
                w3g, w3gv = gt("w3g", bufs=2)
                o3g, o3gv = gt("o3g", bufs=2)
                for n in range(4):
                    cn = L3[n]
                    sl = (slice(None), slice(0, q), slice(7 * n, 7 * n + 7))
                    scsl = (slice(None), slice(0, q),
                            slice(28 + 7 * n, 28 + 7 * n + 7))
                    nc.vector.affine_then_add(w3gv[sl], rsgv[scsl], p3gv[sl],
                                              scale=cn["A"], bias=cn["beta"])
                    nc.vector.scalar_tensor_tensor(
                        o3gv[sl], t3gv[sl], cn["gR"], w3gv[sl], mult, addop)

                # ---- level 2 ----
                # children of node m are o3 cols (2m, 2m+1).  L2/L1 tensors
                # are m-major ([m, q, 7] flat) so per-m slices are contiguous
                # (strided TENSOR_SCALAR is 4x slower and loses 2x mode).
                qf14 = q * 14
                qf7 = q * 7
                l2in = o3g[:].rearrange("p (q m c) -> p m q c", m=2, c=14)
                asl2 = (slice(None), slice(None), slice(0, q), slice(0, 7))
                bsl2 = (slice(None), slice(None), slice(0, q), slice(7, 14))
                sc2, sc2v = gt("sc2")
                sc2m = sc2[:].rearrange("p (m q c) -> p m q c", m=2, c=7)
                nc.gpsimd.tensor_add(sc2m, l2in[asl2], l2in[bsl2])
                p2, p2v = gt("p2")
                p2m = p2[:].rearrange("p (m q c) -> p m q c", m=2, c=7)
                nc.gpsimd.tensor_mul(p2m, l2in[asl2], l2in[bsl2])
                k2, _ = gt("k2")
                nc.scalar.activation(k2[:, 0:qf14], sc2[:, 0:qf14], Ident,
                                     bias=MAGIC, scale=1.0)
                f2, _ = gt("f2")
                nc.vector.scalar_tensor_tensor(f2[:, 0:qf14], k2[:, 0:qf14],
                                               MAGIC, sc2[:, 0:qf14], sub, sub)
                t2, t2v = gt("t2")
                nc.scalar.activation(t2[:, 0:qf14], f2[:, 0:qf14], Sin,
                                     bias=0.0, scale=NEG2PI)
                w2, w2v = gt("w2")
                o2, o2v = gt("o2")
                for m in range(2):
                    cm = L2[m]
                    sl = slice(m * qf7, (m + 1) * qf7)
                    # p2 <- p2*Pg + beta on Pool frees DVE of the affine
                    nc.gpsimd.tensor_scalar(p2[:, sl], p2[:, sl], cm["Pg"],
                                            cm["beta"], mult, addop)
                    nc.vector.scalar_tensor_tensor(
                        w2[:, sl], sc2[:, sl], cm["A"], p2[:, sl], mult, addop)
                    nc.vector.scalar_tensor_tensor(
                        o2[:, sl], t2[:, sl], cm["gR"], w2[:, sl], mult, addop)

                # ---- level 1 ----
                c1 = L1[0]
                sc1, sc1v = gt("sc1")
                nc.gpsimd.tensor_add(sc1[:, 0:qf7], o2[:, 0:qf7],
                                     o2[:, qf7:2 * qf7])
                p1, p1v = gt("p1")
                nc.gpsimd.tensor_mul(p1[:, 0:qf7], o2[:, 0:qf7],
                                     o2[:, qf7:2 * qf7])
                k1, _ = gt("k1")
                nc.scalar.activation(k1[:, 0:qf7], sc1[:, 0:qf7], Ident,
                                     bias=MAGIC, scale=1.0)
                f1, _ = gt("f1")
                nc.vector.scalar_tensor_tensor(f1[:, 0:qf7], k1[:, 0:qf7],
                                               MAGIC, sc1[:, 0:qf7], sub, sub)
                t1, t1v = gt("t1")
                nc.scalar.activation(t1[:, 0:qf7], f1[:, 0:qf7], Sin,
                                     bias=0.0, scale=NEG2PI)
                w1, w1v = gt("w1")
                nc.vector.tensor_scalar(w1[:, 0:qf7], sc1[:, 0:qf7], c1["A"],
                                        c1["beta"], mult, addop)
                nc.vector.scalar_tensor_tensor(
                    w1[:, 0:qf7], p1[:, 0:qf7], c1["Pg"], w1[:, 0:qf7],
                    mult, addop)
                yo, yov = gt("yo")
                nc.vector.scalar_tensor_tensor(
                    yo[:, 0:qf7], t1[:, 0:qf7], c1["gR"], w1[:, 0:qf7],
                    mult, addop)

                dst = out_d[st0:st0 + glen].transpose([1, 0, 2, 3])
                yo4 = yo[:, 0:qf7].rearrange("p (g b a) -> p g b a",
                                             g=glen, a=SLOTS)
                nc.sync.dma_start(dst, yo4)
                st0 += glen

    nc.compile()
    _PROGRAM_CACHE[key] = nc
    return nc


def kernel(x, leaf_w, leaf_b, w1, b1, om1, w2, b2, om2, w3, b3, om3):
    from concourse.bass_interp import get_hw_module
    from concourse.bass_utils import run_bass_kernel_spmd

    L3, L2, L1, wp = _fold(leaf_w, leaf_b, w1, b1, om1, w2, b2, om2, w3, b3, om3)
    _build_program.consts = (L3, L2, L1)
    nc = _build_program()

    in_maps = []
    x = np.ascontiguousarray(x, dtype=F32)
    for c in range(N_CORES):
        xh = _pack_x(x[c * N_CORE:(c + 1) * N_CORE])
        in_maps.append({"xh": xh, "wp": wp})

    kw = {}
    if os.environ.get("KERNEL_TRACE_DIR"):
        kw["tmpdir"] = os.environ["KERNEL_TRACE_DIR"]
    old = nc.m
    nc.m = get_hw_module(nc.m)
    try:
        res = run_bass_kernel_spmd(nc, in_maps, core_ids=list(range(N_CORES)), **kw)
    finally:
        nc.m = old
    kernel._last = res

    out = np.empty(N_FULL, F32)
    for c in range(N_CORES):
        oc = res.results[c]["out"]          # [N_ST, 128, B, 7]
        oc = oc.transpose(0, 2, 1, 3).reshape(-1)[:N_CORE]
        out[c * N_CORE:(c + 1) * N_CORE] = oc
    return out
